# revision 35
# baseline (speedup 1.0000x reference)
"""Trainium2 Bass kernel for causal attention with xpos rotary embedding.

Reference computation (B=1, S=2048, D=2048, H=16 heads, dq=128):
    qkv = x @ w_qkv ; split into per-head q,k,v
    rope(q, scale), rope(k, 1/scale)  (xpos, first 32 dims of each head)
    causal softmax attention per head
    out = concat_heads @ w_out

Sharding: tensor-parallel over heads — each of the 8 cores gets 2 heads
(w_qkv column slice, w_out row slice), computes a full [S, D] partial of
the output projection; host sums the 8 partials (the "all-reduce").

Device kernel strategy (per core, bf16 data / f32 psum accumulate):
  One flat Tile region; phases interleaved per chunk so the scheduler can
  overlap PE (matmuls), ACT (exp), DVE (rope/drains), DMA.
  Phase A chunk c (variable width CK in {256,512} cols of x^T):
    qT/kT = w^T @ x^T per head (PE, bf16, psum [128,512] ring);
    rope on the first 32 rows: rotate-half via two partition-permuted
    SBUF->SBUF DMAs (sign folded into the host sin tables), then
    3 tensor_tensor ops on DVE (host cos/sin tables).
    v computed directly in [S, dq] layout (x-tile stationary).
  Phase B chunk (q-tiles [qt0,qt1), after A covers its key span): per
  head, per ki: scoresT = kT_tile.T @ qT (PE); the causal triangle mask
  is accumulated into the same psum group as a second matmul
  (-BIG * strict-upper @ I), so exp (ACT, sm_scale folded) feeds PV
  directly; PV + ones-denominator accumulate (PE). Denominator
  reciprocal via DVE reciprocal_approx_fast; normalize folded into the
  PV psum->SBUF drain (DVE).
  Phase C (per q-tile, right after its B chunk): y = sum_h OT_h^T @ w_out
  (PE, psum rides the score ring) -> psum drained to bf16 SBUF (DVE/ACT
  alternating) -> one DMA per q-tile row block. Host sums 8 bf16
  partials in f32.
"""
import sys
sys.path.insert(0, "/opt/trn_rl_repo")
import numpy as np

S = 2048
D = 2048
NH = 16
DQ = 128
NROT = 32
SCALE_BASE = 512.0
NCORES = 8
HPC = NH // NCORES          # heads per core = 2
OCN = 3 * HPC               # 6 col tiles of 128 per core (q0,k0,q1,k1,v0,v1)
ST = S // 128               # 16 tiles
QC = 512                    # phase-B max q-chunk width
DC = 512                    # phase-C d-chunk width
SM_SCALE = float(1.0 / np.sqrt(DQ))
MASK_NEG = -904.0           # pre-scale; * SM_SCALE ~= -80 on the logits

# A-phase chunks: (k-tile start, k-tile count). Narrow edges let B/C start
# early and shrink the serial tail after the last A chunk.
A_CHUNKS = [(0, 2), (2, 2), (4, 4), (8, 4), (12, 2), (14, 2)]
# B/C sub-chunk list emitted after A chunk idx: q-tile ranges [qt0, qt1)
BC_AFTER = {1: [(0, 4)], 2: [(4, 8)], 3: [(8, 12)], 4: [(12, 14)], 5: [(14, 15), (15, 16)]}

_CACHE = {}


def _build_program(repeat=1, bench=False):
    import concourse.bacc as bacc
    import concourse.tile as tile
    import concourse.mybir as mybir

    from concourse.bass import AP as bass_AP
    f32 = mybir.dt.float32
    bf16 = mybir.dt.bfloat16
    AF = mybir.ActivationFunctionType
    OP = mybir.AluOpType

    nc = bacc.Bacc("TRN2", target_bir_lowering=False, debug=False,
                   enable_asserts=False, num_devices=NCORES)

    xT_d = nc.dram_tensor("xT", [D, S], bf16, kind="ExternalInput").ap()
    wq_d = nc.dram_tensor("wq", [D, 128 * OCN], bf16, kind="ExternalInput").ap()
    wo_d = nc.dram_tensor("wo", [HPC * DQ, D], bf16, kind="ExternalInput").ap()
    tabs_d = nc.dram_tensor("tabs", [NROT, 4, S], bf16, kind="ExternalInput").ap()
    # packed [trn | id | ones], one DMA (each dma_start costs ~625ns of HWDGE)
    c3_d = nc.dram_tensor("c3", [128, 3, 128], bf16, kind="ExternalInput").ap()
    if bench:
        y_d = nc.dram_tensor("y", [S, D], bf16, kind="Internal").ap()
        ysm_d = nc.dram_tensor("ysm", [128, 64], bf16, kind="ExternalOutput").ap()
    else:
        y_d = nc.dram_tensor("y", [S, D], bf16, kind="ExternalOutput").ap()

    xT_r = xT_d.rearrange("(t p) s -> p t s", p=128)
    wq_r = wq_d.rearrange("(t p) f -> p t f", p=128)

    with tile.TileContext(nc) as tc, \
         tc.tile_pool(name="consts", bufs=1) as consts, \
         tc.tile_pool(name="persist", bufs=1) as persist, \
         tc.tile_pool(name="xtp", bufs=3) as xtp, \
         tc.tile_pool(name="rotp", bufs=3) as rotp, \
         tc.tile_pool(name="pp", bufs=4) as pp, \
         tc.tile_pool(name="recp", bufs=2) as recp, \
         tc.tile_pool(name="ysp", bufs=2) as ysp, \
         tc.tile_pool(name="psA", bufs=2, space="PSUM") as psA, \
         tc.tile_pool(name="psS", bufs=2, space="PSUM") as psS, \
         tc.tile_pool(name="psOT", bufs=1, space="PSUM") as psOT, \
         tc.tile_pool(name="psDen", bufs=1, space="PSUM") as psDen:
      # ---------------- weights/consts: loaded ONCE, resident across reps
      # (weight-stationary). Activations (xT) stream per rep; y stores per
      # rep. First-rep startup interleaves wq pieces with xt0 pieces so A0
      # makes progress as tiles land; each dma_start costs ~625ns of HWDGE.
      ck0 = 128 * A_CHUNKS[0][1]
      wq_sb = persist.tile([128, ST, 128 * OCN], bf16, tag="wq_sb")
      xt0_first = xtp.tile([128, ST, QC], bf16, name="xt0_r0", tag="xt")
      xt0_pieces = [(0, 4), (4, 10), (10, 16)]
      for g, (lo, hi) in enumerate([(0, 3), (3, 8), (8, 16)]):
            nc.sync.dma_start(wq_sb[:, lo:hi, 0:512], wq_r[:, lo:hi, 0:512])
            tl = xt0_pieces[g]
            nc.sync.dma_start(xt0_first[:, tl[0]:tl[1], 0:ck0],
                              xT_r[:, tl[0]:tl[1], 0:ck0])
      for lo, hi in ((0, 8), (8, 16)):
            nc.sync.dma_start(wq_sb[:, lo:hi, 512:768], wq_r[:, lo:hi, 512:768])
      tabs = consts.tile([NROT, 4, S], bf16, tag="tabs")
      nc.sync.dma_start(tabs, tabs_d)
      c3 = consts.tile([128, 3, 128], bf16, tag="c3")
      nc.sync.dma_start(c3, c3_d)
      trn, idm, ones = c3[:, 0, :], c3[:, 1, :], c3[:, 2, :]
      wo_sb = persist.tile([128, HPC, D], bf16, tag="wo_sb")
      nc.sync.dma_start(wo_sb, wo_d.rearrange("(h p) f -> p h f", p=128))

      for rep in range(repeat):
            # qT/kT/vn/OT alternate between two address sets per rep so
            # rep r+1's phase A never waits on rep r's readers (bench path;
            # repeat=1 only ever uses parity 0)
            pr = rep % 2

            xts = {}
            if rep == 0:
                xts[0] = xt0_first
            else:
                xts[0] = xtp.tile([128, ST, QC], bf16, name=f"xt0_r{rep}", tag="xt")
                nc.sync.dma_start(xts[0][:, :, 0:ck0], xT_r[:, :, 0:ck0])
            for c in (1,):
                kt0, ktn = A_CHUNKS[c]
                xts[c] = xtp.tile([128, ST, QC], bf16, name=f"xt{c}_r{rep}", tag="xt")
                nc.sync.dma_start(xts[c][:, :, 0:128 * ktn],
                                  xT_r[:, :, kt0 * 128:(kt0 + ktn) * 128])

            qT = [persist.tile([128, S], bf16, name=f"qT{h}", tag=f"qT{h}_{pr}") for h in range(HPC)]
            kT = [persist.tile([128, S], bf16, name=f"kT{h}", tag=f"kT{h}_{pr}") for h in range(HPC)]
            vn = [persist.tile([128, ST, 128], bf16, name=f"vn{h}", tag=f"vn{h}_{pr}") for h in range(HPC)]
            OT = [persist.tile([128, S], bf16, name=f"OT{h}", tag=f"OT{h}_{pr}") for h in range(HPC)]

            def emit_A_rope(dst, cs, ck, kind):
                # rope on rows 0:NROT, in place. rotate-half via ONE
                # partition-permuted sbuf->sbuf DMA: source partitions in
                # order [16..31, 0..15] (negative outer stride), sign folded
                # into the host sin tables; then 3 DVE tensor_tensor ops.
                rot = rotp.tile([NROT, QC], bf16, tag="rotsb")
                half = NROT // 2
                nc.sync.dma_start(rot[0:half, 0:ck], dst[half:NROT, cs])
                nc.sync.dma_start(rot[half:NROT, 0:ck], dst[0:half, cs])
                ti = 0 if kind == 0 else 2
                nc.vector.tensor_tensor(
                    out=dst[0:NROT, cs], in0=dst[0:NROT, cs],
                    in1=tabs[:, ti, cs], op=OP.mult)
                nc.vector.tensor_tensor(
                    out=rot[:, 0:ck], in0=rot[:, 0:ck],
                    in1=tabs[:, ti + 1, cs], op=OP.mult)
                nc.vector.tensor_tensor(
                    out=dst[0:NROT, cs], in0=dst[0:NROT, cs],
                    in1=rot[:, 0:ck], op=OP.add)

            def emit_A_chunk(kt0, ktn, xt, tmajor=False):
                ck = 128 * ktn
                cs = slice(kt0 * 128, kt0 * 128 + ck)
                if tmajor:
                    # DMA-feed-limited window (chunk 0): keep all 4 qk groups
                    # open (2 halves of each [128,512] psA tile) and stream
                    # t-major so PE consumes each wq/xt piece as it lands.
                    assert ck == 256
                    tiles = [psA.tile([128, QC], f32, name=f"tm{i}_r{rep}", tag="qkv")
                             for i in range(2)]
                    # two groups share each bank: the start bit pending-zeroes
                    # the WHOLE 2KB bank, so explicitly memset and accumulate
                    # with start=False throughout.
                    for i in range(2):
                        nc.vector.memset(tiles[i], 0.0)
                    for t in range(ST):
                        for oc in (0, 2, 1, 3):
                            ps = tiles[oc // 2]
                            co = 256 * (oc % 2)
                            nc.tensor.matmul(
                                ps[:, co:co + ck],
                                wq_sb[:, t, oc * 128:(oc + 1) * 128],
                                xt[:, t, 0:ck], start=False,
                                stop=(t == ST - 1), skip_group_check=True)
                    for oc in (0, 2, 1, 3):
                        h, kind = divmod(oc, 2)
                        ps = tiles[oc // 2]
                        co = 256 * (oc % 2)
                        dst = qT[h] if kind == 0 else kT[h]
                        if kind == 0:
                            nc.vector.tensor_copy(dst[:, cs], ps[:, co:co + ck])
                        else:
                            nc.scalar.copy(dst[:, cs], ps[:, co:co + ck])
                        emit_A_rope(dst, cs, ck, kind)
                else:
                    for oc in (0, 2, 1, 3):     # q_h0, q_h1, k_h0, k_h1
                        h, kind = divmod(oc, 2)
                        ps = psA.tile([128, QC], f32, tag="qkv")
                        for t in range(ST):
                            nc.tensor.matmul(
                                ps[:, 0:ck], wq_sb[:, t, oc * 128:(oc + 1) * 128],
                                xt[:, t, 0:ck], start=(t == 0), stop=(t == ST - 1))
                        dst = qT[h] if kind == 0 else kT[h]
                        if kind == 0:
                            nc.vector.tensor_copy(dst[:, cs], ps[:, 0:ck])
                        else:
                            nc.scalar.copy(dst[:, cs], ps[:, 0:ck])
                        emit_A_rope(dst, cs, ck, kind)
                # v directly in [s, dq] layout, both heads' cols at once;
                # two s-tiles share one [128, 512] psA ring tile.
                for sp2 in range((ktn + 1) // 2):
                    pv = psA.tile([128, QC], f32, tag="qkv")
                    for u in range(min(2, ktn - 2 * sp2)):
                        si = 2 * sp2 + u
                        st = kt0 + si
                        for t in range(ST):
                            nc.tensor.matmul(
                                pv[:, 256 * u:256 * (u + 1)],
                                xt[:, t, si * 128:(si + 1) * 128],
                                wq_sb[:, t, 4 * 128:6 * 128],
                                start=(t == 0), stop=(t == ST - 1))
                        nc.vector.tensor_copy(vn[0][:, st, :], pv[:, 256 * u:256 * u + 128])
                        nc.scalar.copy(vn[1][:, st, :], pv[:, 256 * u + 128:256 * (u + 1)])

            def emit_B_chunk(qt0, qt1):
                # attention for q-tiles [qt0, qt1): history keys ki < qt0 at
                # full width, diagonal keys ki in [qt0, qt1) on the live
                # q-subrange only; the diag triangle mask is a second matmul
                # accumulated into the scores psum group.
                w = 128 * (qt1 - qt0)
                q0 = 128 * qt0
                gsz = (2 * QC) // w     # history k-tiles per exp group
                for h in range(HPC):
                    ot = psOT.tile([128, QC], f32, tag="ot")
                    dn = psDen.tile([128, QC], f32, tag="dn")
                    # full-width history tiles (ki < qt0) first (their kT/qT
                    # deps are ready before the diag's same-chunk kT rope)
                    for kg in range(0, qt0, gsz):
                        ng = min(gsz, qt0 - kg)
                        sp = psS.tile([128, 2 * QC], f32, tag="sc")
                        for u in range(ng):
                            ki = kg + u
                            nc.tensor.matmul(
                                sp[:, u * w:(u + 1) * w],
                                kT[h][:, ki * 128:(ki + 1) * 128],
                                qT[h][:, q0:q0 + w], start=True, stop=True)
                        p = pp.tile([128, 2 * QC], bf16, tag="p")
                        nc.scalar.activation(p[:, 0:ng * w], sp[:, 0:ng * w],
                                             AF.Exp, scale=SM_SCALE)
                        for u in range(ng):
                            st = (kg == 0 and u == 0)
                            nc.tensor.matmul(
                                ot[:, 0:w], vn[h][:, kg + u, :],
                                p[:, u * w:(u + 1) * w],
                                start=st, stop=False, skip_group_check=True)
                            nc.tensor.matmul(
                                dn[:, 0:w], ones, p[:, u * w:(u + 1) * w],
                                start=st, stop=False, skip_group_check=True)
                    # diagonal tiles last
                    for m in range(qt1 - qt0):
                        ki = qt0 + m
                        qoff = 128 * m
                        sp = psS.tile([128, 2 * QC], f32, tag="sc")
                        nc.tensor.matmul(
                            sp[:, qoff:w], kT[h][:, ki * 128:(ki + 1) * 128],
                            qT[h][:, q0 + qoff:q0 + w], start=True, stop=False,
                            skip_group_check=True)
                        nc.tensor.matmul(
                            sp[:, qoff:qoff + 128], trn, idm,
                            start=False, stop=True, skip_group_check=True)
                        p = pp.tile([128, 2 * QC], bf16, tag="p")
                        nc.scalar.activation(p[:, qoff:w], sp[:, qoff:w],
                                             AF.Exp, scale=SM_SCALE)
                        st = (qt0 == 0 and m == 0)
                        last = (m == qt1 - qt0 - 1)
                        nc.tensor.matmul(
                            ot[:, qoff:w], vn[h][:, ki, :], p[:, qoff:w],
                            start=st, stop=last, skip_group_check=True)
                        nc.tensor.matmul(
                            dn[:, qoff:w], ones, p[:, qoff:w],
                            start=st, stop=last, skip_group_check=True)
                    rec = recp.tile([128, QC], f32, tag="rec")
                    nc.vector.reciprocal_approx_fast(rec[:, 0:w], dn[:, 0:w])
                    nc.vector.tensor_tensor(
                        out=OT[h][:, q0:q0 + w], in0=ot[:, 0:w],
                        in1=rec[:, 0:w], op=OP.mult)

            def emit_C_chunk(qt0, qt1):
                # yp psum rides the psS ring ([128, 2, QC] = two DC-wide chunks)
                for qt in range(qt0, qt1):
                    ys = ysp.tile([128, D], bf16, tag="ys")
                    for dc2 in range(D // (2 * DC)):
                        yp = psS.tile([128, 2 * QC], f32, tag="sc")
                        for u in range(2):
                            ds_ = slice((2 * dc2 + u) * DC, (2 * dc2 + u + 1) * DC)
                            for h in range(HPC):
                                nc.tensor.matmul(
                                    yp[:, u * QC:(u + 1) * QC],
                                    OT[h][:, qt * 128:(qt + 1) * 128],
                                    wo_sb[:, h, ds_],
                                    start=(h == 0), stop=(h == HPC - 1))
                            if (dc2 + u) % 2 == 0:
                                nc.vector.tensor_copy(ys[:, ds_], yp[:, u * QC:(u + 1) * QC])
                            else:
                                nc.scalar.copy(ys[:, ds_], yp[:, u * QC:(u + 1) * QC])
                    # one store per tile; the last two tiles flush in
                    # pieces so the final drain->store tail is short
                    nsp = 4 if qt == ST - 1 else (2 if qt == ST - 2 else 1)
                    for sp_ in range(nsp):
                        dsl = slice(sp_ * D // nsp, (sp_ + 1) * D // nsp)
                        nc.scalar.dma_start(y_d[qt * 128:(qt + 1) * 128, dsl],
                                            ys[:, dsl])
                    if bench and qt == ST - 1:
                        nc.scalar.dma_start(ysm_d, ys[:, 0:64])

            for c, (kt0, ktn) in enumerate(A_CHUNKS):
                pf = c + 2
                if pf < len(A_CHUNKS) and pf not in xts:
                    pkt0, pktn = A_CHUNKS[pf]
                    xts[pf] = xtp.tile([128, ST, QC], bf16, name=f"xt{pf}_r{rep}", tag="xt")
                    nc.sync.dma_start(
                        xts[pf][:, :, 0:128 * pktn],
                        xT_r[:, :, pkt0 * 128:(pkt0 + pktn) * 128])
                emit_A_chunk(kt0, ktn, xts[c], tmajor=False)
                for qt0, qt1 in BC_AFTER.get(c, []):
                    emit_B_chunk(qt0, qt1)
                    emit_C_chunk(qt0, qt1)
    nc.compile()
    return nc


def _host_tables():
    """cos/sin xpos tables, computed in fp32 mirroring the jax reference.
    Rows 0:16 of the sin tables are NEGATED: rotate-half's first half is
    -t2, and the device applies rot = permuted(t) * sin_table."""
    t = np.arange(S, dtype=np.float32)
    inv_freq = (1.0 / (10000.0 ** (np.arange(0, NROT, 2, dtype=np.float32) / NROT))
                ).astype(np.float32)
    freqs = t[:, None] * inv_freq[None, :]              # [S, 16]
    pos = np.concatenate([freqs, freqs], axis=-1)       # [S, 32]
    base_scale = ((np.arange(0, NROT, 2, dtype=np.float32) + 0.4 * NROT)
                  / (1.4 * NROT)).astype(np.float32)
    power = (t - S // 2) / np.float32(SCALE_BASE)
    scale = base_scale[None, :] ** power[:, None]       # [S, 16]
    scale = np.concatenate([scale, scale], axis=-1)     # [S, 32]
    cos, sin = np.cos(pos), np.sin(pos)
    rscale = (np.float32(1.0) / scale).astype(np.float32)
    sgn = np.ones((1, NROT), np.float32)
    sgn[0, 0:NROT // 2] = -1.0
    tabs = np.stack([
        (cos * scale).T, (sin * scale * sgn).T,         # q: cq, sq
        (cos * rscale).T, (sin * rscale * sgn).T,       # k: ck, sk
    ], axis=1).astype(np.float32)                       # [32, 4, S]
    return np.ascontiguousarray(tabs)


def _host_consts():
    tabs = _host_tables()
    # diag mask as matmul: scores[kl, ql] += MASK_NEG where ql < kl.
    # out = trn^T @ I -> out[a, b] = trn[b, a]; want out[kl, ql] = MASK_NEG
    # for ql < kl -> trn[ql, kl] = MASK_NEG for kl > ql (strict upper).
    trn = np.triu(np.full((128, 128), MASK_NEG, np.float32), k=1)
    idm = np.eye(128, dtype=np.float32)
    ones = np.ones((128, 128), dtype=np.float32)
    c3 = np.ascontiguousarray(np.stack([trn, idm, ones], axis=1))  # [128,3,128]
    return tabs, c3


def _get_runner(repeat=1, bench=False):
    key = ("runner", repeat, bench)
    if key not in _CACHE:
        from runner_embedded import BassRunner
        nc = _build_program(repeat, bench=bench)
        _CACHE[key] = BassRunner(nc, n_cores=NCORES, donate=False)
    return _CACHE[key]


def make_in_maps(x, w_qkv, w_out):
    import ml_dtypes
    bf = ml_dtypes.bfloat16
    x = np.asarray(x, dtype=np.float32)
    w_qkv = np.asarray(w_qkv, dtype=np.float32)
    w_out = np.asarray(w_out, dtype=np.float32)
    xT = np.ascontiguousarray(x.reshape(S, D).T.astype(bf))
    tabs, c3 = _host_consts()
    tabs, c3 = tabs.astype(bf), c3.astype(bf)
    in_maps = []
    for c in range(NCORES):
        wslice = w_qkv[:, c * 128 * OCN:(c + 1) * 128 * OCN]
        # reference layout per head: [q(128)|k(128)|v(128)]; device layout:
        # [q0,k0,q1,k1,v0,v1]
        wq = np.concatenate([
            wslice[:, 0:128], wslice[:, 128:256],       # q0, k0
            wslice[:, 384:512], wslice[:, 512:640],     # q1, k1
            wslice[:, 256:384], wslice[:, 640:768],     # v0, v1
        ], axis=1)
        in_maps.append({
            "xT": xT,
            "wq": np.ascontiguousarray(wq.astype(bf)),
            "wo": np.ascontiguousarray(
                w_out[c * HPC * DQ:(c + 1) * HPC * DQ, :].astype(bf)),
            "tabs": tabs, "c3": c3,
        })
    return in_maps


def kernel(x, w_qkv, w_out):
    runner = _get_runner(repeat=1)
    in_maps = make_in_maps(x, w_qkv, w_out)
    results = runner(in_maps)
    y = np.zeros((S, D), dtype=np.float32)
    for c in range(NCORES):
        y += results[c]["y"].astype(np.float32)
    return y.reshape(1, S, D)


# ---------------------------------------------------------------------------
# Embedded PJRT runner (kernel.py must be self-contained).
import importlib.util as _ilu
import types as _types

_runner_src = '''
import sys
sys.path.insert(0, "/opt/trn_rl_repo")
import time
import numpy as np
import jax
import jax.numpy as jnp
from jax.experimental.shard_map import shard_map
from jax.sharding import Mesh, PartitionSpec

import concourse.mybir as mybir
from concourse.bass2jax import install_neuronx_cc_hook, _bass_exec_p, partition_id_tensor


class BassRunner:
    def __init__(self, nc, n_cores=8, donate=True):
        install_neuronx_cc_hook()
        self.nc = nc
        self.n_cores = n_cores
        self.donate = donate

        partition_name = nc.partition_id_tensor.name if nc.partition_id_tensor else None
        in_names, out_names, out_avals, zero_outs = [], [], [], []
        for alloc in nc.m.functions[0].allocations:
            if not isinstance(alloc, mybir.MemoryLocationSet):
                continue
            name = alloc.memorylocations[0].name
            if alloc.kind == "ExternalInput":
                if name != partition_name:
                    in_names.append(name)
            elif alloc.kind == "ExternalOutput":
                out_names.append(name)
                shape = tuple(alloc.tensor_shape)
                dtype = mybir.dt.np(alloc.dtype)
                out_avals.append(jax.core.ShapedArray(shape, dtype))
                zero_outs.append(np.zeros(shape, dtype))
        self.in_names = list(in_names)
        self.out_names = out_names
        self.out_avals = out_avals
        self.zero_outs = zero_outs
        n_params = len(in_names)
        n_outs = len(out_avals)
        all_in_names = list(in_names) + list(out_names)
        if partition_name is not None:
            all_in_names.append(partition_name)

        def _body(*args):
            operands = list(args)
            if partition_name is not None:
                operands.append(partition_id_tensor())
            outs = _bass_exec_p.bind(
                *operands,
                out_avals=tuple(out_avals),
                in_names=tuple(all_in_names),
                out_names=tuple(out_names),
                lowering_input_output_aliases=(),
                sim_require_finite=True,
                sim_require_nnan=True,
                nc=nc,
            )
            return tuple(outs)

        devices = jax.devices()[:n_cores]
        assert len(devices) == n_cores
        self.mesh = Mesh(np.asarray(devices), ("core",))
        in_specs = (PartitionSpec("core"),) * (n_params + n_outs)
        out_specs = (PartitionSpec("core"),) * n_outs
        donate_argnums = tuple(range(n_params, n_params + n_outs)) if donate else ()
        self.fn = jax.jit(
            shard_map(_body, mesh=self.mesh, in_specs=in_specs,
                      out_specs=out_specs, check_rep=False),
            donate_argnums=donate_argnums, keep_unused=True,
        )
        self.n_params = n_params
        self.n_outs = n_outs

    def concat_inputs(self, in_maps):
        return [
            np.concatenate([np.asarray(in_maps[c][name]) for c in range(self.n_cores)], axis=0)
            for name in self.in_names
        ]

    def __call__(self, in_maps):
        concat_in = self.concat_inputs(in_maps)
        concat_zeros = [
            np.zeros((self.n_cores * z.shape[0], *z.shape[1:]), z.dtype)
            for z in self.zero_outs
        ]
        out_arrs = self.fn(*concat_in, *concat_zeros)
        return [
            {name: np.asarray(out_arrs[i]).reshape(self.n_cores, *self.out_avals[i].shape)[c]
             for i, name in enumerate(self.out_names)}
            for c in range(self.n_cores)
        ]

    def sharded_inputs(self, in_maps):
        from jax.sharding import NamedSharding
        sh = NamedSharding(self.mesh, PartitionSpec("core"))
        concat_in = [jax.device_put(x, sh) for x in self.concat_inputs(in_maps)]
        concat_zeros = [
            jax.device_put(np.zeros((self.n_cores * z.shape[0], *z.shape[1:]), z.dtype), sh)
            for z in self.zero_outs
        ]
        return concat_in, concat_zeros

    def bench(self, in_maps, reps=10, warmup=2):
        assert not self.donate
        concat_in, concat_zeros = self.sharded_inputs(in_maps)
        times = []
        for i in range(reps + warmup):
            t0 = time.perf_counter()
            out = self.fn(*concat_in, *concat_zeros)
            jax.block_until_ready(out)
            dt = time.perf_counter() - t0
            if i >= warmup:
                times.append(dt)
        return times

    def bench_pipelined(self, in_maps, batch=20, warmup=3):
        assert not self.donate
        concat_in, concat_zeros = self.sharded_inputs(in_maps)
        for _ in range(warmup):
            jax.block_until_ready(self.fn(*concat_in, *concat_zeros))
        outs = []
        t0 = time.perf_counter()
        for _ in range(batch):
            outs.append(self.fn(*concat_in, *concat_zeros))
        jax.block_until_ready(outs)
        return (time.perf_counter() - t0) / batch
'''

_spec = _ilu.spec_from_loader("runner_embedded", loader=None)
_mod = _types.ModuleType("runner_embedded")
exec(_runner_src, _mod.__dict__)
sys.modules["runner_embedded"] = _mod


# revision 43
# speedup vs baseline: 1.0308x; 1.0308x over previous
"""Trainium2 Bass kernel for causal attention with xpos rotary embedding.

Reference computation (B=1, S=2048, D=2048, H=16 heads, dq=128):
    qkv = x @ w_qkv ; split into per-head q,k,v
    rope(q, scale), rope(k, 1/scale)  (xpos, first 32 dims of each head)
    causal softmax attention per head
    out = concat_heads @ w_out

Sharding: tensor-parallel over heads — each of the 8 cores gets 2 heads
(w_qkv column slice, w_out row slice), computes a full [S, D] partial of
the output projection; host sums the 8 partials (the "all-reduce").

Device kernel strategy (per core, bf16 data / f32 psum accumulate):
  One flat Tile region; phases interleaved per chunk so the scheduler can
  overlap PE (matmuls), ACT (exp), DVE (rope/drains), DMA.
  Phase A chunk c (variable width CK in {256,512} cols of x^T):
    qT/kT = w^T @ x^T per head (PE, bf16, psum [128,512] ring);
    rope on the first 32 rows: rotate-half via two partition-permuted
    SBUF->SBUF DMAs (sign folded into the host sin tables), then
    3 tensor_tensor ops on DVE (host cos/sin tables).
    v computed directly in [S, dq] layout (x-tile stationary).
  Phase B chunk (q-tiles [qt0,qt1), after A covers its key span): per
  head, per ki: scoresT = kT_tile.T @ qT (PE); the causal triangle mask
  is accumulated into the same psum group as a second matmul
  (-BIG * strict-upper @ I), so exp (ACT, sm_scale folded) feeds PV
  directly; PV + ones-denominator accumulate (PE). Denominator
  reciprocal via DVE reciprocal_approx_fast; normalize folded into the
  PV psum->SBUF drain (DVE).
  Phase C (per q-tile, right after its B chunk): y = sum_h OT_h^T @ w_out
  (PE, psum rides the score ring) -> psum drained to bf16 SBUF (DVE/ACT
  alternating) -> one DMA per q-tile row block. Host sums 8 bf16
  partials in f32.
"""
import sys
sys.path.insert(0, "/opt/trn_rl_repo")
import numpy as np

S = 2048
D = 2048
NH = 16
DQ = 128
NROT = 32
SCALE_BASE = 512.0
NCORES = 8
HPC = NH // NCORES          # heads per core = 2
OCN = 3 * HPC               # 6 col tiles of 128 per core (q0,k0,q1,k1,v0,v1)
ST = S // 128               # 16 tiles
QC = 512                    # phase-B max q-chunk width
DC = 512                    # phase-C d-chunk width
SM_SCALE = float(1.0 / np.sqrt(DQ))
MASK_NEG = -904.0           # pre-scale; * SM_SCALE ~= -80 on the logits

# A-phase chunks: (k-tile start, k-tile count). Narrow edges let B/C start
# early and shrink the serial tail after the last A chunk.
A_CHUNKS = [(0, 2), (2, 2), (4, 4), (8, 4), (12, 2), (14, 2)]
# B/C sub-chunk list emitted after A chunk idx: q-tile ranges [qt0, qt1)
BC_AFTER = {1: [(0, 4)], 2: [(4, 8)], 3: [(8, 12)], 4: [(12, 14)], 5: [(14, 16)]}

_CACHE = {}


def _build_program(repeat=1, bench=False):
    import concourse.bacc as bacc
    import concourse.tile as tile
    import concourse.mybir as mybir

    f32 = mybir.dt.float32
    bf16 = mybir.dt.bfloat16
    AF = mybir.ActivationFunctionType
    OP = mybir.AluOpType

    nc = bacc.Bacc("TRN2", target_bir_lowering=False, debug=False,
                   enable_asserts=False, num_devices=NCORES)

    xT_d = nc.dram_tensor("xT", [D, S], bf16, kind="ExternalInput").ap()
    wq_d = nc.dram_tensor("wq", [D, 128 * OCN], bf16, kind="ExternalInput").ap()
    wo_d = nc.dram_tensor("wo", [HPC * DQ, D], bf16, kind="ExternalInput").ap()
    tabs_d = nc.dram_tensor("tabs", [NROT, 4, S], bf16, kind="ExternalInput").ap()
    # packed [trn | id | ones], one DMA (each dma_start costs ~625ns of HWDGE)
    c3_d = nc.dram_tensor("c3", [128, 3, 128], bf16, kind="ExternalInput").ap()
    if bench:
        y_d = nc.dram_tensor("y", [S, D], bf16, kind="Internal").ap()
        ysm_d = nc.dram_tensor("ysm", [128, 64], bf16, kind="ExternalOutput").ap()
    else:
        y_d = nc.dram_tensor("y", [S, D], bf16, kind="ExternalOutput").ap()

    xT_r = xT_d.rearrange("(t p) s -> p t s", p=128)
    wq_r = wq_d.rearrange("(t p) f -> p t f", p=128)

    with tile.TileContext(nc) as tc, \
         tc.tile_pool(name="consts", bufs=1) as consts, \
         tc.tile_pool(name="persist", bufs=1) as persist, \
         tc.tile_pool(name="xtp", bufs=3) as xtp, \
         tc.tile_pool(name="rotp", bufs=3) as rotp, \
         tc.tile_pool(name="pp", bufs=4) as pp, \
         tc.tile_pool(name="recp", bufs=2) as recp, \
         tc.tile_pool(name="ysp", bufs=2) as ysp, \
         tc.tile_pool(name="psA", bufs=2, space="PSUM") as psA, \
         tc.tile_pool(name="psS", bufs=2, space="PSUM") as psS, \
         tc.tile_pool(name="psOT", bufs=1, space="PSUM") as psOT, \
         tc.tile_pool(name="psDen", bufs=1, space="PSUM") as psDen:
      # ---------------- weights/consts: loaded ONCE, resident across reps
      # (weight-stationary). Activations (xT) stream per rep; y stores per
      # rep. First-rep startup interleaves wq pieces with xt0 pieces so A0
      # makes progress as tiles land; each dma_start costs ~625ns of HWDGE.
      ck0 = 128 * A_CHUNKS[0][1]
      wq_sb = persist.tile([128, ST, 128 * OCN], bf16, tag="wq_sb")
      xt0_first = xtp.tile([128, ST, QC], bf16, name="xt0_r0", tag="xt")
      xt0_pieces = [(0, 4), (4, 10), (10, 16)]
      for g, (lo, hi) in enumerate([(0, 3), (3, 8), (8, 16)]):
            nc.sync.dma_start(wq_sb[:, lo:hi, 0:512], wq_r[:, lo:hi, 0:512])
            tl = xt0_pieces[g]
            nc.sync.dma_start(xt0_first[:, tl[0]:tl[1], 0:ck0],
                              xT_r[:, tl[0]:tl[1], 0:ck0])
      for lo, hi in ((0, 8), (8, 16)):
            nc.sync.dma_start(wq_sb[:, lo:hi, 512:768], wq_r[:, lo:hi, 512:768])
      tabs = consts.tile([NROT, 4, S], bf16, tag="tabs")
      nc.sync.dma_start(tabs, tabs_d)
      c3 = consts.tile([128, 3, 128], bf16, tag="c3")
      nc.sync.dma_start(c3, c3_d)
      trn, idm, ones = c3[:, 0, :], c3[:, 1, :], c3[:, 2, :]
      wo_sb = persist.tile([128, HPC, D], bf16, tag="wo_sb")
      nc.sync.dma_start(wo_sb, wo_d.rearrange("(h p) f -> p h f", p=128))

      for rep in range(repeat):
            # qT/kT/vn/OT alternate between two address sets per rep so
            # rep r+1's phase A never waits on rep r's readers (bench path;
            # repeat=1 only ever uses parity 0)
            pr = rep % 2

            xts = {}
            if rep == 0:
                xts[0] = xt0_first
            else:
                xts[0] = xtp.tile([128, ST, QC], bf16, name=f"xt0_r{rep}", tag="xt")
                nc.sync.dma_start(xts[0][:, :, 0:ck0], xT_r[:, :, 0:ck0])
            for c in (1,):
                kt0, ktn = A_CHUNKS[c]
                xts[c] = xtp.tile([128, ST, QC], bf16, name=f"xt{c}_r{rep}", tag="xt")
                nc.sync.dma_start(xts[c][:, :, 0:128 * ktn],
                                  xT_r[:, :, kt0 * 128:(kt0 + ktn) * 128])

            qT = [persist.tile([128, S], bf16, name=f"qT{h}", tag=f"qT{h}_{pr}") for h in range(HPC)]
            kT = [persist.tile([128, S], bf16, name=f"kT{h}", tag=f"kT{h}_{pr}") for h in range(HPC)]
            vn = [persist.tile([128, ST, 128], bf16, name=f"vn{h}", tag=f"vn{h}_{pr}") for h in range(HPC)]
            OT = [persist.tile([128, S], bf16, name=f"OT{h}", tag=f"OT{h}_{pr}") for h in range(HPC)]

            def emit_A_rope(dst, cs, ck, kind):
                # rope on rows 0:NROT, in place. rotate-half via two
                # partition-shifted sbuf->sbuf DMAs (sign folded into the
                # host sin tables), then 3 DVE tensor_tensor ops.
                rot = rotp.tile([NROT, QC], bf16, tag="rotsb")
                half = NROT // 2
                nc.sync.dma_start(rot[0:half, 0:ck], dst[half:NROT, cs])
                nc.sync.dma_start(rot[half:NROT, 0:ck], dst[0:half, cs])
                ti = 0 if kind == 0 else 2
                nc.vector.tensor_tensor(
                    out=dst[0:NROT, cs], in0=dst[0:NROT, cs],
                    in1=tabs[:, ti, cs], op=OP.mult)
                nc.vector.tensor_tensor(
                    out=rot[:, 0:ck], in0=rot[:, 0:ck],
                    in1=tabs[:, ti + 1, cs], op=OP.mult)
                nc.vector.tensor_tensor(
                    out=dst[0:NROT, cs], in0=dst[0:NROT, cs],
                    in1=rot[:, 0:ck], op=OP.add)

            def emit_A_chunk(kt0, ktn, xt, tmajor=False):
                ck = 128 * ktn
                cs = slice(kt0 * 128, kt0 * 128 + ck)
                if tmajor:
                    # DMA-feed-limited window (chunk 0): keep all 4 qk groups
                    # open (2 halves of each [128,512] psA tile) and stream
                    # t-major so PE consumes each wq/xt piece as it lands.
                    assert ck == 256
                    tiles = [psA.tile([128, QC], f32, name=f"tm{i}_r{rep}", tag="qkv")
                             for i in range(2)]
                    # two groups share each bank: the start bit pending-zeroes
                    # the WHOLE 2KB bank, so explicitly memset and accumulate
                    # with start=False throughout.
                    for i in range(2):
                        nc.vector.memset(tiles[i], 0.0)
                    for t in range(ST):
                        for oc in (0, 2, 1, 3):
                            ps = tiles[oc // 2]
                            co = 256 * (oc % 2)
                            nc.tensor.matmul(
                                ps[:, co:co + ck],
                                wq_sb[:, t, oc * 128:(oc + 1) * 128],
                                xt[:, t, 0:ck], start=False,
                                stop=(t == ST - 1), skip_group_check=True)
                    for oc in (0, 2, 1, 3):
                        h, kind = divmod(oc, 2)
                        ps = tiles[oc // 2]
                        co = 256 * (oc % 2)
                        dst = qT[h] if kind == 0 else kT[h]
                        if kind == 0:
                            nc.vector.tensor_copy(dst[:, cs], ps[:, co:co + ck])
                        else:
                            nc.scalar.copy(dst[:, cs], ps[:, co:co + ck])
                        emit_A_rope(dst, cs, ck, kind)
                else:
                    for oc in (0, 2, 1, 3):     # q_h0, q_h1, k_h0, k_h1
                        h, kind = divmod(oc, 2)
                        ps = psA.tile([128, QC], f32, tag="qkv")
                        for t in range(ST):
                            nc.tensor.matmul(
                                ps[:, 0:ck], wq_sb[:, t, oc * 128:(oc + 1) * 128],
                                xt[:, t, 0:ck], start=(t == 0), stop=(t == ST - 1))
                        dst = qT[h] if kind == 0 else kT[h]
                        if kind == 0:
                            nc.vector.tensor_copy(dst[:, cs], ps[:, 0:ck])
                        else:
                            nc.scalar.copy(dst[:, cs], ps[:, 0:ck])
                        emit_A_rope(dst, cs, ck, kind)
                # v directly in [s, dq] layout, both heads' cols at once;
                # two s-tiles share one [128, 512] psA ring tile.
                for sp2 in range((ktn + 1) // 2):
                    pv = psA.tile([128, QC], f32, tag="qkv")
                    for u in range(min(2, ktn - 2 * sp2)):
                        si = 2 * sp2 + u
                        st = kt0 + si
                        for t in range(ST):
                            nc.tensor.matmul(
                                pv[:, 256 * u:256 * (u + 1)],
                                xt[:, t, si * 128:(si + 1) * 128],
                                wq_sb[:, t, 4 * 128:6 * 128],
                                start=(t == 0), stop=(t == ST - 1))
                        nc.vector.tensor_copy(vn[0][:, st, :], pv[:, 256 * u:256 * u + 128])
                        nc.scalar.copy(vn[1][:, st, :], pv[:, 256 * u + 128:256 * (u + 1)])

            def emit_B_chunk(qt0, qt1):
                # attention for q-tiles [qt0, qt1): history keys ki < qt0 at
                # full width, diagonal keys ki in [qt0, qt1) on the live
                # q-subrange only; the diag triangle mask is a second matmul
                # accumulated into the scores psum group.
                w = 128 * (qt1 - qt0)
                q0 = 128 * qt0
                gsz = (2 * QC) // w     # history k-tiles per exp group
                for h in range(HPC):
                    ot = psOT.tile([128, QC], f32, tag="ot")
                    dn = psDen.tile([128, QC], f32, tag="dn")
                    # full-width history tiles (ki < qt0) first (their kT/qT
                    # deps are ready before the diag's same-chunk kT rope)
                    for kg in range(0, qt0, gsz):
                        ng = min(gsz, qt0 - kg)
                        sp = psS.tile([128, 2 * QC], f32, tag="sc")
                        for u in range(ng):
                            ki = kg + u
                            nc.tensor.matmul(
                                sp[:, u * w:(u + 1) * w],
                                kT[h][:, ki * 128:(ki + 1) * 128],
                                qT[h][:, q0:q0 + w], start=True, stop=True)
                        p = pp.tile([128, 2 * QC], bf16, tag="p")
                        nc.scalar.activation(p[:, 0:ng * w], sp[:, 0:ng * w],
                                             AF.Exp, scale=SM_SCALE)
                        for u in range(ng):
                            st = (kg == 0 and u == 0)
                            nc.tensor.matmul(
                                ot[:, 0:w], vn[h][:, kg + u, :],
                                p[:, u * w:(u + 1) * w],
                                start=st, stop=False, skip_group_check=True)
                            nc.tensor.matmul(
                                dn[:, 0:w], ones, p[:, u * w:(u + 1) * w],
                                start=st, stop=False, skip_group_check=True)
                    # diagonal tiles last
                    for m in range(qt1 - qt0):
                        ki = qt0 + m
                        qoff = 128 * m
                        sp = psS.tile([128, 2 * QC], f32, tag="sc")
                        nc.tensor.matmul(
                            sp[:, qoff:w], kT[h][:, ki * 128:(ki + 1) * 128],
                            qT[h][:, q0 + qoff:q0 + w], start=True, stop=False,
                            skip_group_check=True)
                        nc.tensor.matmul(
                            sp[:, qoff:qoff + 128], trn, idm,
                            start=False, stop=True, skip_group_check=True)
                        p = pp.tile([128, 2 * QC], bf16, tag="p")
                        nc.scalar.activation(p[:, qoff:w], sp[:, qoff:w],
                                             AF.Exp, scale=SM_SCALE)
                        st = (qt0 == 0 and m == 0)
                        last = (m == qt1 - qt0 - 1)
                        nc.tensor.matmul(
                            ot[:, qoff:w], vn[h][:, ki, :], p[:, qoff:w],
                            start=st, stop=last, skip_group_check=True)
                        nc.tensor.matmul(
                            dn[:, qoff:w], ones, p[:, qoff:w],
                            start=st, stop=last, skip_group_check=True)
                    rec = recp.tile([128, QC], f32, tag="rec")
                    nc.vector.reciprocal_approx_fast(rec[:, 0:w], dn[:, 0:w])
                    nc.vector.tensor_tensor(
                        out=OT[h][:, q0:q0 + w], in0=ot[:, 0:w],
                        in1=rec[:, 0:w], op=OP.mult)

            def emit_C_chunk(qt0, qt1):
                # yp psum rides the psS ring ([128, 2, QC] = two DC-wide chunks)
                for qt in range(qt0, qt1):
                    ys = ysp.tile([128, D], bf16, tag="ys")
                    for dc2 in range(D // (2 * DC)):
                        yp = psS.tile([128, 2 * QC], f32, tag="sc")
                        for u in range(2):
                            ds_ = slice((2 * dc2 + u) * DC, (2 * dc2 + u + 1) * DC)
                            for h in range(HPC):
                                nc.tensor.matmul(
                                    yp[:, u * QC:(u + 1) * QC],
                                    OT[h][:, qt * 128:(qt + 1) * 128],
                                    wo_sb[:, h, ds_],
                                    start=(h == 0), stop=(h == HPC - 1))
                            if (dc2 + u) % 2 == 0:
                                nc.vector.tensor_copy(ys[:, ds_], yp[:, u * QC:(u + 1) * QC])
                            else:
                                nc.scalar.copy(ys[:, ds_], yp[:, u * QC:(u + 1) * QC])
                    # one store per tile; the last two tiles flush in
                    # pieces so the final drain->store tail is short
                    nsp = 4 if qt == ST - 1 else (2 if qt == ST - 2 else 1)
                    for sp_ in range(nsp):
                        dsl = slice(sp_ * D // nsp, (sp_ + 1) * D // nsp)
                        nc.sync.dma_start(y_d[qt * 128:(qt + 1) * 128, dsl],
                                          ys[:, dsl])
                    if bench and qt == ST - 1:
                        nc.sync.dma_start(ysm_d, ys[:, 0:64])

            for c, (kt0, ktn) in enumerate(A_CHUNKS):
                pf = c + 2
                if pf < len(A_CHUNKS) and pf not in xts:
                    pkt0, pktn = A_CHUNKS[pf]
                    xts[pf] = xtp.tile([128, ST, QC], bf16, name=f"xt{pf}_r{rep}", tag="xt")
                    nc.sync.dma_start(
                        xts[pf][:, :, 0:128 * pktn],
                        xT_r[:, :, pkt0 * 128:(pkt0 + pktn) * 128])
                emit_A_chunk(kt0, ktn, xts[c], tmajor=False)
                for qt0, qt1 in BC_AFTER.get(c, []):
                    emit_B_chunk(qt0, qt1)
                    emit_C_chunk(qt0, qt1)
    nc.compile()
    return nc


def _host_tables():
    """cos/sin xpos tables, computed in fp32 mirroring the jax reference.
    Rows 0:16 of the sin tables are NEGATED: rotate-half's first half is
    -t2, and the device applies rot = permuted(t) * sin_table."""
    t = np.arange(S, dtype=np.float32)
    inv_freq = (1.0 / (10000.0 ** (np.arange(0, NROT, 2, dtype=np.float32) / NROT))
                ).astype(np.float32)
    freqs = t[:, None] * inv_freq[None, :]              # [S, 16]
    pos = np.concatenate([freqs, freqs], axis=-1)       # [S, 32]
    base_scale = ((np.arange(0, NROT, 2, dtype=np.float32) + 0.4 * NROT)
                  / (1.4 * NROT)).astype(np.float32)
    power = (t - S // 2) / np.float32(SCALE_BASE)
    scale = base_scale[None, :] ** power[:, None]       # [S, 16]
    scale = np.concatenate([scale, scale], axis=-1)     # [S, 32]
    cos, sin = np.cos(pos), np.sin(pos)
    rscale = (np.float32(1.0) / scale).astype(np.float32)
    sgn = np.ones((1, NROT), np.float32)
    sgn[0, 0:NROT // 2] = -1.0
    tabs = np.stack([
        (cos * scale).T, (sin * scale * sgn).T,         # q: cq, sq
        (cos * rscale).T, (sin * rscale * sgn).T,       # k: ck, sk
    ], axis=1).astype(np.float32)                       # [32, 4, S]
    return np.ascontiguousarray(tabs)


def _host_consts():
    tabs = _host_tables()
    # diag mask as matmul: scores[kl, ql] += MASK_NEG where ql < kl.
    # out = trn^T @ I -> out[a, b] = trn[b, a]; want out[kl, ql] = MASK_NEG
    # for ql < kl -> trn[ql, kl] = MASK_NEG for kl > ql (strict upper).
    trn = np.triu(np.full((128, 128), MASK_NEG, np.float32), k=1)
    idm = np.eye(128, dtype=np.float32)
    ones = np.ones((128, 128), dtype=np.float32)
    c3 = np.ascontiguousarray(np.stack([trn, idm, ones], axis=1))  # [128,3,128]
    return tabs, c3


def _get_runner(repeat=1, bench=False):
    key = ("runner", repeat, bench)
    if key not in _CACHE:
        from runner_embedded import BassRunner
        nc = _build_program(repeat, bench=bench)
        _CACHE[key] = BassRunner(nc, n_cores=NCORES, donate=False)
    return _CACHE[key]


def make_in_maps(x, w_qkv, w_out):
    import ml_dtypes
    bf = ml_dtypes.bfloat16
    x = np.asarray(x, dtype=np.float32)
    w_qkv = np.asarray(w_qkv, dtype=np.float32)
    w_out = np.asarray(w_out, dtype=np.float32)
    xT = np.ascontiguousarray(x.reshape(S, D).T.astype(bf))
    tabs, c3 = _host_consts()
    tabs, c3 = tabs.astype(bf), c3.astype(bf)
    in_maps = []
    for c in range(NCORES):
        wslice = w_qkv[:, c * 128 * OCN:(c + 1) * 128 * OCN]
        # reference layout per head: [q(128)|k(128)|v(128)]; device layout:
        # [q0,k0,q1,k1,v0,v1]
        wq = np.concatenate([
            wslice[:, 0:128], wslice[:, 128:256],       # q0, k0
            wslice[:, 384:512], wslice[:, 512:640],     # q1, k1
            wslice[:, 256:384], wslice[:, 640:768],     # v0, v1
        ], axis=1)
        in_maps.append({
            "xT": xT,
            "wq": np.ascontiguousarray(wq.astype(bf)),
            "wo": np.ascontiguousarray(
                w_out[c * HPC * DQ:(c + 1) * HPC * DQ, :].astype(bf)),
            "tabs": tabs, "c3": c3,
        })
    return in_maps


def kernel(x, w_qkv, w_out):
    runner = _get_runner(repeat=1)
    in_maps = make_in_maps(x, w_qkv, w_out)
    results = runner(in_maps)
    y = np.zeros((S, D), dtype=np.float32)
    for c in range(NCORES):
        y += results[c]["y"].astype(np.float32)
    return y.reshape(1, S, D)


# ---------------------------------------------------------------------------
# Embedded PJRT runner (kernel.py must be self-contained).
import importlib.util as _ilu
import types as _types

_runner_src = '''
import sys
sys.path.insert(0, "/opt/trn_rl_repo")
import time
import numpy as np
import jax
import jax.numpy as jnp
from jax.experimental.shard_map import shard_map
from jax.sharding import Mesh, PartitionSpec

import concourse.mybir as mybir
from concourse.bass2jax import install_neuronx_cc_hook, _bass_exec_p, partition_id_tensor


class BassRunner:
    def __init__(self, nc, n_cores=8, donate=True):
        install_neuronx_cc_hook()
        self.nc = nc
        self.n_cores = n_cores
        self.donate = donate

        partition_name = nc.partition_id_tensor.name if nc.partition_id_tensor else None
        in_names, out_names, out_avals, zero_outs = [], [], [], []
        for alloc in nc.m.functions[0].allocations:
            if not isinstance(alloc, mybir.MemoryLocationSet):
                continue
            name = alloc.memorylocations[0].name
            if alloc.kind == "ExternalInput":
                if name != partition_name:
                    in_names.append(name)
            elif alloc.kind == "ExternalOutput":
                out_names.append(name)
                shape = tuple(alloc.tensor_shape)
                dtype = mybir.dt.np(alloc.dtype)
                out_avals.append(jax.core.ShapedArray(shape, dtype))
                zero_outs.append(np.zeros(shape, dtype))
        self.in_names = list(in_names)
        self.out_names = out_names
        self.out_avals = out_avals
        self.zero_outs = zero_outs
        n_params = len(in_names)
        n_outs = len(out_avals)
        all_in_names = list(in_names) + list(out_names)
        if partition_name is not None:
            all_in_names.append(partition_name)

        def _body(*args):
            operands = list(args)
            if partition_name is not None:
                operands.append(partition_id_tensor())
            outs = _bass_exec_p.bind(
                *operands,
                out_avals=tuple(out_avals),
                in_names=tuple(all_in_names),
                out_names=tuple(out_names),
                lowering_input_output_aliases=(),
                sim_require_finite=True,
                sim_require_nnan=True,
                nc=nc,
            )
            return tuple(outs)

        devices = jax.devices()[:n_cores]
        assert len(devices) == n_cores
        self.mesh = Mesh(np.asarray(devices), ("core",))
        in_specs = (PartitionSpec("core"),) * (n_params + n_outs)
        out_specs = (PartitionSpec("core"),) * n_outs
        donate_argnums = tuple(range(n_params, n_params + n_outs)) if donate else ()
        self.fn = jax.jit(
            shard_map(_body, mesh=self.mesh, in_specs=in_specs,
                      out_specs=out_specs, check_rep=False),
            donate_argnums=donate_argnums, keep_unused=True,
        )
        self.n_params = n_params
        self.n_outs = n_outs

    def concat_inputs(self, in_maps):
        return [
            np.concatenate([np.asarray(in_maps[c][name]) for c in range(self.n_cores)], axis=0)
            for name in self.in_names
        ]

    def __call__(self, in_maps):
        concat_in = self.concat_inputs(in_maps)
        concat_zeros = [
            np.zeros((self.n_cores * z.shape[0], *z.shape[1:]), z.dtype)
            for z in self.zero_outs
        ]
        out_arrs = self.fn(*concat_in, *concat_zeros)
        return [
            {name: np.asarray(out_arrs[i]).reshape(self.n_cores, *self.out_avals[i].shape)[c]
             for i, name in enumerate(self.out_names)}
            for c in range(self.n_cores)
        ]

    def sharded_inputs(self, in_maps):
        from jax.sharding import NamedSharding
        sh = NamedSharding(self.mesh, PartitionSpec("core"))
        concat_in = [jax.device_put(x, sh) for x in self.concat_inputs(in_maps)]
        concat_zeros = [
            jax.device_put(np.zeros((self.n_cores * z.shape[0], *z.shape[1:]), z.dtype), sh)
            for z in self.zero_outs
        ]
        return concat_in, concat_zeros

    def bench(self, in_maps, reps=10, warmup=2):
        assert not self.donate
        concat_in, concat_zeros = self.sharded_inputs(in_maps)
        times = []
        for i in range(reps + warmup):
            t0 = time.perf_counter()
            out = self.fn(*concat_in, *concat_zeros)
            jax.block_until_ready(out)
            dt = time.perf_counter() - t0
            if i >= warmup:
                times.append(dt)
        return times

    def bench_pipelined(self, in_maps, batch=20, warmup=3):
        assert not self.donate
        concat_in, concat_zeros = self.sharded_inputs(in_maps)
        for _ in range(warmup):
            jax.block_until_ready(self.fn(*concat_in, *concat_zeros))
        outs = []
        t0 = time.perf_counter()
        for _ in range(batch):
            outs.append(self.fn(*concat_in, *concat_zeros))
        jax.block_until_ready(outs)
        return (time.perf_counter() - t0) / batch
'''

_spec = _ilu.spec_from_loader("runner_embedded", loader=None)
_mod = _types.ModuleType("runner_embedded")
exec(_runner_src, _mod.__dict__)
sys.modules["runner_embedded"] = _mod
